# revision 33
# baseline (speedup 1.0000x reference)
"""Trainium2 Bass kernel for GQA attention (nn_Attention_50053548868012).

Math (reference):
  q = einsum('bsm,mrkh->brksh', x, wq);  k = x@wk;  v = x@wv   (per kv head)
  RoPE on q, k; causal-masked softmax(q k^T / sqrt(H)); y = a @ v;
  out = einsum('brksh,rkhm->bsm', y, wo)

Sharding: tensor-parallel over the KV-head axis - core c owns kv head c
(its 4 query heads, wk/wv column slices, and the 512-row slice of wo).
Each core computes its partial full-size output; the host sums the 8
partial outputs (the all-reduce).

v2 design (vs the phase-separated v1 baseline):
 - everything bf16 (rel err ~7e-3 vs the 2e-2 gate): halves DMA and SBUF
   so all weights stay resident and x is read exactly once.
 - chunk-major fusion: for each 512-seq chunk, projections accumulate
   32-deep in PSUM (no DVE spill-adds), RoPE runs on DVE under the next
   chunk's PE work, then attention + the output projection for the chunk
   keep the PE stream dense end-to-end.
 - softmax denominator: e-tiles are accumulated on DVE (bf16 2x) and
   reduced with ONE ones-matmul per (chunk, head) instead of a second
   full matmul pipe on the PE (-80k PE rows).
 - exp is the only table function on ACT; spills/copies shared between
   ACT engine queues so no engine's in-order queue blocks another
   segment's critical path.
"""

import numpy as np
import ml_dtypes

import concourse.bass as bass
import concourse.tile as tile
from concourse import bacc, mybir
from concourse.bass_utils import run_bass_kernel_spmd
from concourse.masks import make_identity

NCORES = 8
S = 2048
MD = 4096
H = 128
R = 4
KV = 8
PT = 128           # partition tile
SC = 512           # seq chunk = matmul free dim
NMT = MD // PT     # 32 model-dim tiles
MB = 8             # m-tiles per x/wq slab
NSL = NMT // MB    # 4 slabs
TPC = SC // PT     # 4 seq-tiles per chunk
HH = H // 2
RH = R * H         # 512
SCALE = float(H) ** -0.5
NEG = -30000.0

f32 = mybir.dt.float32
bf16 = mybir.dt.bfloat16
BF_NP = ml_dtypes.bfloat16

import os
UNIT_OUTER = os.environ.get("K_UNIT_OUTER", "1") == "1"

EXP = mybir.ActivationFunctionType.Exp


def build_bass(s=S, collective=True, phases=3, reps=1):
    nc = _emit(s, phases, reps)
    nc.compile()
    return nc


def _emit(s, phases, reps):
    n_sc = s // SC
    n_tt = s // PT
    nc = bacc.Bacc("TRN2", target_bir_lowering=False, debug=False,
                   num_devices=NCORES)

    xc = nc.dram_tensor("xc", [PT, n_sc, NMT, SC], bf16,
                        kind="ExternalInput").ap()
    wqd = nc.dram_tensor("wq", [PT, NSL, MB, RH], bf16,
                         kind="ExternalInput").ap()
    wkd = nc.dram_tensor("wk", [PT, NMT, H], bf16, kind="ExternalInput").ap()
    wvd = nc.dram_tensor("wv", [PT, NMT, H], bf16, kind="ExternalInput").ap()
    wod = nc.dram_tensor("wo", [PT, R, MD], bf16, kind="ExternalInput").ap()
    cosd = nc.dram_tensor("cosT", [H, s], bf16, kind="ExternalInput").ap()
    sind = nc.dram_tensor("sinT", [H, s], bf16, kind="ExternalInput").ap()
    maskd = nc.dram_tensor("mask4", [PT, TPC * SC], f32,
                           kind="ExternalInput").ap()
    outp = nc.dram_tensor("outp", [PT, n_tt, MD], bf16,
                          kind="ExternalOutput").ap()

    with tile.TileContext(nc) as tc:
      for _rep in range(reps):
        with tc.tile_pool(name="const", bufs=1) as cpool, \
             tc.tile_pool(name="wts", bufs=1) as wpool, \
             tc.tile_pool(name="seqst", bufs=1) as spool, \
             tc.tile_pool(name="xslab", bufs=5) as xpool, \
             tc.tile_pool(name="qy", bufs=2) as qypool, \
             tc.tile_pool(name="ep", bufs=4) as epool, \
             tc.tile_pool(name="small", bufs=2) as smpool, \
             tc.tile_pool(name="oacc", bufs=2) as opool:

            # ---------------- weights + consts ----------------
            wq_sb = wpool.tile([PT, NMT, RH], bf16)
            wk_sb = wpool.tile([PT, NMT, H], bf16)
            wv_sb = wpool.tile([PT, NMT, H], bf16)
            wo_sb = wpool.tile([PT, R, MD], bf16)
            nc.gpsimd.dma_start(wk_sb[:], wkd)
            nc.gpsimd.dma_start(wv_sb[:], wvd)
            cos_sb = cpool.tile([H, s], bf16)
            nc.gpsimd.dma_start(cos_sb[:], cosd)
            sin_sb = cpool.tile([H, s], bf16)
            nc.gpsimd.dma_start(sin_sb[:], sind)
            mask_sb = cpool.tile([PT, TPC, SC], f32)
            nc.gpsimd.dma_start(
                mask_sb[:], maskd.rearrange("p (j c) -> p j c", j=TPC))
            nc.gpsimd.dma_start(wo_sb[:], wod)
            ones_bf = cpool.tile([PT, PT], bf16)
            nc.gpsimd.memset(ones_bf[:], 1.0)
            ident = cpool.tile([PT, PT], bf16)
            make_identity(nc, ident[:])

            # per-chunk k/v tiles: separate tiles so chunk c's RoPE/transpose
            # writes create no false deps against reads of older chunks
            kT_cs = [spool.tile([H, SC], bf16, name=f"kT{i}")
                     for i in range(n_sc)]
            v_cs = [spool.tile([PT, TPC, H], bf16, name=f"v{i}")
                    for i in range(n_sc)]

            pend = []  # deferred (z / finalize) emitters

            def drain_one():
                if pend:
                    pend.pop(0)()

            def drain_all():
                while pend:
                    pend.pop(0)()

            # ---------------- phase 1: projections + RoPE ----------------
            def ph1(c, xs=None, skip_v=False):
                csl = slice(c * SC, (c + 1) * SC)
                first = xs is None
                with tc.tile_pool(name=f"p1ps{c}", bufs=1,
                                  space="PSUM") as pp, \
                     tc.tile_pool(name=f"tpps{c}", bufs=2,
                                  space="PSUM") as tpp:
                    nu = R + 1 if skip_v else R + 2
                    ps_u = [pp.tile([PT, SC], f32, tag=f"u{u}",
                                    name=f"ps_u{u}") for u in range(nu)]
                    qT_c = qypool.tile([H, R, SC], bf16, tag="qt",
                                       name=f"qT{c}")
                    vT_c = (None if skip_v else
                            smpool.tile([H, SC], bf16, tag="vt",
                                        name=f"vT{c}"))
                    qsw = smpool.tile([H, R, SC], bf16, tag="qsw", bufs=1,
                                      name=f"qsw{c}")
                    ksw = smpool.tile([H, SC], bf16, tag="ksw", bufs=1,
                                      name=f"ksw{c}")

                    def wsl(u, m):
                        if u < R:
                            return wq_sb[:, m, u * H:(u + 1) * H]
                        if u == R:
                            return wk_sb[:, m, :]
                        return wv_sb[:, m, :]

                    def spill(u):
                        if u < R:
                            nc.scalar.copy(qT_c[:, u, :], ps_u[u][:])
                            if u == R - 1:
                                # all q heads spilled: swap + q-RoPE on DVE
                                nc.gpsimd.dma_start(qsw[0:HH, :, :],
                                                    qT_c[HH:H, :, :])
                                nc.gpsimd.dma_start(qsw[HH:H, :, :],
                                                    qT_c[0:HH, :, :])
                                sin_c = sin_sb[:, csl][:, None, :] \
                                    .broadcast_to([H, R, SC])
                                cos_c = cos_sb[:, csl][:, None, :] \
                                    .broadcast_to([H, R, SC])
                                nc.vector.tensor_mul(qsw[:], qsw[:], sin_c)
                                nc.vector.tensor_mul(qT_c[:], qT_c[:],
                                                     cos_c)
                                nc.vector.tensor_add(qT_c[:], qT_c[:],
                                                     qsw[:])
                        elif u == R:
                            kT_c = kT_cs[c]
                            nc.scalar.copy(kT_c[:], ps_u[R][:])
                            nc.gpsimd.dma_start(ksw[0:HH, :], kT_c[HH:H, :])
                            nc.gpsimd.dma_start(ksw[HH:H, :], kT_c[0:HH, :])
                            nc.vector.tensor_mul(ksw[:], ksw[:],
                                                 sin_sb[:, csl])
                            nc.vector.tensor_mul(kT_c[:], kT_c[:],
                                                 cos_sb[:, csl])
                            nc.vector.tensor_add(kT_c[:], kT_c[:], ksw[:])
                        else:
                            nc.scalar.copy(vT_c[:], ps_u[R + 1][:])

                    def transposes():
                        for tt in range(TPC):
                            ps_t = tpp.tile([PT, PT], bf16, tag="tp",
                                            name="ps_t")
                            nc.tensor.transpose(
                                ps_t[:], vT_c[:, tt * PT:(tt + 1) * PT],
                                ident[:])
                            nc.scalar.copy(v_cs[c][:, tt, :], ps_t[:])

                    if first:  # chunk 0: slab-outer hides the x/wq DMA ramp
                        xs = []
                        for sl in range(NSL):
                            xsl = xpool.tile([PT, MB, SC], bf16, tag="x",
                                             name=f"x{c}_{sl}")
                            nc.sync.dma_start(
                                xsl[:], xc[:, c, sl * MB:(sl + 1) * MB, :])
                            nc.sync.dma_start(
                                wq_sb[:, sl * MB:(sl + 1) * MB, :],
                                wqd[:, sl, :, :])
                            xs.append(xsl)
                        for sl in range(NSL):
                            for ml in range(MB):
                                m = sl * MB + ml
                                rx = xs[sl][:, ml, :]
                                st = m == 0
                                sp = m == NMT - 1
                                # k/v first: their weights land before wq
                                for u in (R, R + 1, 0, 1, 2, 3):
                                    nc.tensor.matmul(ps_u[u][:], wsl(u, m),
                                                     rx, start=st, stop=sp)
                        spill(R + 1)
                        for u in range(R + 1):
                            spill(u)
                        transposes()
                    elif UNIT_OUTER:  # unit-outer, spill per unit
                        def unit_mms(u):
                            for m in range(NMT):
                                nc.tensor.matmul(
                                    ps_u[u][:], wsl(u, m),
                                    xs[m // MB][:, m % MB, :],
                                    start=(m == 0), stop=(m == NMT - 1))

                        if not skip_v:
                            unit_mms(R + 1)               # v
                            spill(R + 1)
                        unit_mms(0)                       # q0
                        if not skip_v:
                            transposes()                  # behind q0 mms
                        spill(0)
                        for u in (1, 2, 3, R):            # q1..q3, k
                            unit_mms(u)
                            spill(u)
                    else:  # slab-outer: rotate psum banks every matmul
                        for m in range(NMT):
                            rx = xs[m // MB][:, m % MB, :]
                            st = m == 0
                            sp = m == NMT - 1
                            for u in range(nu):
                                nc.tensor.matmul(ps_u[u][:], wsl(u, m),
                                                 rx, start=st, stop=sp)
                        if not skip_v:
                            spill(R + 1)
                        for u in range(R + 1):
                            spill(u)
                        if not skip_v:
                            transposes()
                return qT_c

            # ---- phase 3 as interleavable work items (PE filler) ----
            def ph3_items(cprev, yT_prev, aps):
                items = []
                oaccs = {}
                for ti in range(TPC):
                    st = cprev * TPC + ti
                    for mc in range(MD // SC):
                        def item(ti=ti, mc=mc, st=st, yT_prev=yT_prev,
                                 aps=aps):
                            if mc == 0:
                                oaccs[ti] = opool.tile(
                                    [PT, MD], bf16, tag="oa",
                                    name=f"oacc{st}")
                            o_acc = oaccs[ti]
                            ps_o = aps.tile([PT, SC], f32, tag="po",
                                            bufs=2, name="ps_o")
                            for rl in range(R):
                                nc.tensor.matmul(
                                    ps_o[:],
                                    yT_prev[:, rl, ti * PT:(ti + 1) * PT],
                                    wo_sb[:, rl, mc * SC:(mc + 1) * SC],
                                    start=(rl == 0), stop=(rl == R - 1))
                            osl = o_acc[:, mc * SC:(mc + 1) * SC]
                            if mc % 2 == 0:
                                nc.scalar.copy(osl, ps_o[:])
                            else:
                                nc.vector.tensor_copy(osl, ps_o[:])
                            if mc == MD // SC - 1:
                                hm = MD // 2
                                nc.sync.dma_start(outp[:, st, 0:hm],
                                                  o_acc[:, 0:hm])
                                nc.gpsimd.dma_start(outp[:, st, hm:MD],
                                                    o_acc[:, hm:MD])
                        items.append(item)
                return items

            # v-projection of a later chunk, chopped into PE-filler items
            # (used to keep the PE busy inside the partner-less attn(0))
            def v_items(c2, aps):
                xs2 = _prefetched[c2]
                box = {}
                its = []
                for g in range(NMT // 4):
                    def item(g=g):
                        if g == 0:
                            box["ps"] = aps.tile([PT, SC], f32, tag="pv",
                                                 bufs=1, name="ps_pv")
                        for ml in range(4):
                            m = 4 * g + ml
                            nc.tensor.matmul(
                                box["ps"][:], wv_sb[:, m, :],
                                xs2[m // MB][:, m % MB, :],
                                start=(m == 0), stop=(m == NMT - 1))
                    its.append(item)

                def fin():
                    vT2 = smpool.tile([H, SC], bf16, tag="vt",
                                      name=f"vT{c2}")
                    nc.scalar.copy(vT2[:], box["ps"][:])
                    for tt in range(TPC):
                        ps_t = aps.tile([PT, PT], bf16, tag="ptp",
                                        bufs=1, name="ps_t2")
                        nc.tensor.transpose(
                            ps_t[:], vT2[:, tt * PT:(tt + 1) * PT],
                            ident[:])
                        nc.scalar.copy(v_cs[c2][:, tt, :], ps_t[:])
                its.append(fin)
                return its

            # -------- A-segment: attention(c) x output-proj(c-1) --------
            def aseg(c, qT_c, yT_prev, xpf=None, pre_v=None):
                T = (c + 1) * TPC
                P = T // 2
                yT_c = qypool.tile([H, R, SC], bf16, tag="yt",
                                   name=f"yT{c}")
                with tc.tile_pool(name=f"aps{c}", bufs=1,
                                  space="PSUM") as aps:
                    if xpf is not None:
                        prefetch(xpf)
                    if yT_prev is not None:
                        items = ph3_items(c - 1, yT_prev, aps)
                    elif pre_v is not None:
                        items = v_items(pre_v, aps)
                    else:
                        items = []
                    n_items = len(items)
                    total_pairs = R * P
                    pairs_done = [0]
                    items_done = [0]
                    # with pre_v, items need the just-started x prefetch;
                    # hold them until the first pair's DMA shadow passes
                    min_pair = 1 if pre_v is not None else 0

                    def drain1():
                        if pend:
                            pend.pop(0)()
                        elif items and pairs_done[0] >= min_pair:
                            items.pop(0)()
                            items_done[0] += 1

                    for j in range(R):
                        ps_y = aps.tile([H, SC], f32, tag="y", bufs=1,
                                        name=f"psy{c}_{j}")
                        esum = smpool.tile([PT, SC], bf16, tag="es",
                                           name=f"es{c}_{j}")
                        es = {}

                        def qk_exp(p, j=j, qT_c=qT_c, T=T, es=es):
                            t0 = 2 * p
                            ps_s = aps.tile([PT, 2 * SC], f32, tag="s",
                                            bufs=2, name="ps_s")
                            def kt(t):
                                return kT_cs[t // TPC][
                                    :, (t % TPC) * PT:(t % TPC + 1) * PT]

                            nc.tensor.matmul(
                                ps_s[:, 0:SC], kt(t0),
                                qT_c[:, j, :], start=True, stop=True)
                            nc.tensor.matmul(
                                ps_s[:, SC:2 * SC], kt(t0 + 1),
                                qT_c[:, j, :], start=True, stop=True)
                            drain1()
                            jj = t0 - (T - TPC)
                            if jj >= 0:
                                nc.vector.tensor_add(
                                    ps_s[:].rearrange(
                                        "q (a b) -> q a b", a=2),
                                    ps_s[:].rearrange(
                                        "q (a b) -> q a b", a=2),
                                    mask_sb[:, jj:jj + 2, :])
                            e_t = epool.tile([PT, 2 * SC], bf16, tag="e",
                                             name="e_t")
                            nc.scalar.activation(e_t[:], ps_s[:], EXP,
                                                 scale=SCALE)
                            es[p] = e_t

                        qk_exp(0)
                        if P > 1:
                            qk_exp(1)
                        for p in range(P):
                            if p + 2 < P:
                                qk_exp(p + 2)
                            e_t = es.pop(p)
                            for half in range(2):
                                t = 2 * p + half
                                esl = slice(half * SC, (half + 1) * SC)
                                nc.tensor.matmul(
                                    ps_y[:], v_cs[t // TPC][:, t % TPC, :],
                                    e_t[:, esl],
                                    start=(t == 0), stop=(t == T - 1))
                                if t == 0:
                                    nc.vector.tensor_copy(esum[:],
                                                          e_t[:, esl])
                                else:
                                    nc.vector.tensor_add(esum[:], esum[:],
                                                         e_t[:, esl])
                            pairs_done[0] += 1
                            # hold back a few items as PE filler for the
                            # final z/recip/broadcast chain
                            target = min(
                                (n_items * pairs_done[0]) // total_pairs,
                                n_items - 3)
                            while items_done[0] < target and items:
                                items.pop(0)()
                                items_done[0] += 1

                        def stage1(esum=esum, aps=aps):
                            ps_z = aps.tile([PT, SC], f32, tag="zb",
                                            bufs=1, name="ps_zb")
                            nc.tensor.matmul(ps_z[0:1, :], ones_bf[:, 0:1],
                                             esum[:], start=True, stop=True)
                            rz = smpool.tile([1, SC], bf16, tag="rz",
                                             name="rz")
                            with nc.allow_low_precision(
                                    reason="bf16 softmax denom, 2e-2 tol"):
                                nc.vector.reciprocal(rz[:], ps_z[0:1, :])
                            return rz

                        rz_box = []

                        def stage2(rz_box=rz_box, ps_y=ps_y, yT_c=yT_c,
                                   j=j, aps=aps):
                            ps_b = aps.tile([PT, SC], f32, tag="zb",
                                            bufs=1, name="ps_zb")
                            nc.tensor.matmul(ps_b[:], ones_bf[0:1, :],
                                             rz_box[0][:], start=True,
                                             stop=True)
                            b_sb = smpool.tile([PT, SC], f32, tag="bb",
                                               name="b_sb")
                            nc.scalar.copy(b_sb[:], ps_b[:])
                            nc.vector.tensor_mul(yT_c[:, j, :], ps_y[:],
                                                 b_sb[:])

                        pend.append(lambda s1=stage1, rb=rz_box: rb.append(
                            s1()))
                        pend.append(stage2)
                    # tail: interleave remaining items between the last
                    # head's z / finalize so the PE never sits idle
                    while pend:
                        pend.pop(0)()
                        if items:
                            items.pop(0)()
                    while items:
                        items.pop(0)()
                return yT_c

            # -------- tail output projection (no attention partner) --------
            def ph3_tail(cprev, yT_prev):
                with tc.tile_pool(name="p3tail", bufs=1,
                                  space="PSUM") as aps:
                    items = ph3_items(cprev, yT_prev, aps)
                    for it in items:
                        it()

            _prefetched = {c: [] for c in range(n_sc)}

            def prefetch(c):
                for sl in range(NSL):
                    xsl = xpool.tile([PT, MB, SC], bf16, tag="x",
                                     name=f"x{c}_{sl}")
                    nc.sync.dma_start(
                        xsl[:], xc[:, c, sl * MB:(sl + 1) * MB, :])
                    _prefetched[c].append(xsl)

            qts = {}
            yts = {}
            qts[0] = ph1(0)
            prefetch(1)
            qts[1] = ph1(1, _prefetched[1])
            yts[0] = aseg(0, qts[0], None, xpf=2, pre_v=2)
            qts[2] = ph1(2, _prefetched[2], skip_v=True)
            yts[1] = aseg(1, qts[1], yts[0], xpf=3)
            qts[3] = ph1(3, _prefetched[3])
            yts[2] = aseg(2, qts[2], yts[1])
            yts[3] = aseg(3, qts[3], yts[2])
            ph3_tail(3, yts[3])
            drain_all()
    return nc


# ---------------------------------------------------------------------------
# host-side packing
# ---------------------------------------------------------------------------

def make_mask4():
    """mask4[:, 512j:512(j+1)][ti, sj] = 0 if 128j+ti <= sj else NEG."""
    m = np.full((PT, TPC * SC), NEG, dtype=np.float32)
    for j in range(TPC):
        ti = np.arange(PT)[:, None]
        sj = np.arange(SC)[None, :]
        m[:, j * SC:(j + 1) * SC] = np.where(128 * j + ti <= sj, 0.0, NEG)
    return m


def shard_inputs(x, wq, wk, wv, wo, mask, sin, cos, s=S):
    del mask  # causality hardcoded via mask4
    n_sc = s // SC
    xT = np.asarray(x, np.float32).reshape(s, MD).T  # [MD, s]
    xc = np.ascontiguousarray(
        xT.reshape(NMT, PT, n_sc, SC).transpose(1, 2, 0, 3)).astype(BF_NP)
    cosT = np.asarray(cos, np.float32).T.astype(BF_NP)
    sign = np.concatenate(
        [-np.ones((HH, 1)), np.ones((HH, 1))]).astype(np.float32)
    sinT = (np.asarray(sin, np.float32).T * sign).astype(BF_NP)
    cosT = np.ascontiguousarray(cosT)
    sinT = np.ascontiguousarray(sinT)
    mask4 = make_mask4()
    wq = np.asarray(wq, np.float32)
    wk = np.asarray(wk, np.float32)
    wv = np.asarray(wv, np.float32)
    wo = np.asarray(wo, np.float32)
    in_maps = []
    for c in range(NCORES):
        wqc = wq[:, :, c, :].reshape(MD, RH)             # [M, R*H]
        wqp = np.ascontiguousarray(
            wqc.reshape(NSL, MB, PT, RH).transpose(2, 0, 1, 3)).astype(BF_NP)
        wkp = np.ascontiguousarray(
            wk[:, c, :].reshape(NMT, PT, H).transpose(1, 0, 2)).astype(BF_NP)
        wvp = np.ascontiguousarray(
            wv[:, c, :].reshape(NMT, PT, H).transpose(1, 0, 2)).astype(BF_NP)
        wop = np.ascontiguousarray(
            wo[:, c, :, :].transpose(1, 0, 2)).astype(BF_NP)  # [H, R, MD]
        in_maps.append({
            "xc": xc, "wq": wqp, "wk": wkp, "wv": wvp, "wo": wop,
            "cosT": cosT, "sinT": sinT, "mask4": mask4,
        })
    return in_maps


def unpack_out(outp_arr, s=S):
    a = np.asarray(outp_arr).astype(np.float32)
    return a.reshape(PT, s // PT, MD).transpose(1, 0, 2).reshape(s, MD)


_NC_CACHE = {}


def kernel(x, wq, wk, wv, wo, mask, sin, cos):
    s = x.shape[1]
    if s not in _NC_CACHE:
        _NC_CACHE[s] = build_bass(s)
    nc = _NC_CACHE[s]
    in_maps = shard_inputs(x, wq, wk, wv, wo, mask, sin, cos, s=s)
    res = run_bass_kernel_spmd(nc, in_maps, list(range(NCORES)))
    out = unpack_out(res.results[0]["outp"], s)
    for c in range(1, NCORES):
        out = out + unpack_out(res.results[c]["outp"], s)
    return out.reshape(1, s, MD).astype(np.float32)
